# revision 1
# baseline (speedup 1.0000x reference)
"""Causal self-attention (B=2, S=2048, D=2048, H=16, HD=128) on 8 TRN2 cores.

Sharding: core c -> batch b = c//4, heads 4*(c%4)..4*(c%4)+3 (tensor-parallel
over heads within a batch; data-parallel over batch across core groups).

v3 design (baseline fp32r ~442us, v2 bf16 ~388us):
  - Everything feeding the PE is bf16 (host converts x/W; q/k/v/p/ctx produced
    in bf16 on-chip). bf16 runs matmuls at 1 cycle/row at every size, where
    fp32r drops to 4 cycles/row below 256 output columns, and halves
    DMA + SBUF footprint.
  - q^T/k^T/v stay resident in SBUF between projection and attention.
  - Batched DMA: one rearranged descriptor per weight matrix / per x s-block
    (the v2 per-chunk scheme burned ~620ns of issue time per DMA on the sync
    queue and starved the PE early in phase 1). Loads are spread across the
    sync/gpsimd/scalar queues so issues overlap.
  - softmax row-sums split across engines to balance load: for half the
    (head, q-block) pairs a ones-column PE matmul accumulates l per k-chunk;
    for the other half DVE accumulates exp chunks and one tiny PE matmul
    reduces the result.
  - output partials are written bf16 (host upconverts + sums); PSUM
    evacuation alternates ACT/DVE.
Emission is software-pipelined: PV/l matmuls lag score matmuls by 2 chunks,
the RoPE shuffle lags the projection group by 1, and finalize/output-
projection jobs are drip-fed into the attention chunk stream.
"""

import math
from collections import deque

import ml_dtypes
import numpy as np

import concourse.bacc as bacc
import concourse.mybir as mybir
from concourse.tile import TileContext
from concourse.bass_utils import run_bass_kernel_spmd

B, S, D = 2, 2048, 2048
H, HD = 16, 128
ROPE_THETA = 10000.0

N_CORES = 8
CORES_PER_BATCH = 4
HPC = H // (N_CORES // B)  # heads per core = 4
HL = HPC * HD              # 512 local head-dim columns
NDC = D // 128             # 16 contraction chunks
NSB = S // 512             # 4 s-blocks
NKC = S // 128             # 16 k-chunks

F32 = mybir.dt.float32
BF16 = mybir.dt.bfloat16
AF = mybir.ActivationFunctionType
BNP = ml_dtypes.bfloat16


def _mm(nc, out, lhsT, rhs, start, stop):
    nc.tensor.matmul(out, lhsT, rhs, start=start, stop=stop)


def _build():
    nc = bacc.Bacc("TRN2", target_bir_lowering=False, debug=False)

    # all big operands arrive pre-rearranged to the on-chip layout so each
    # DMA is one contiguous run per partition (descriptor-fragmented loads
    # measured ~4x slower)
    xT = nc.dram_tensor("xT", [128, NSB, NDC, 512], BF16, kind="ExternalInput")
    wq = nc.dram_tensor("wq", [128, NDC, HL], BF16, kind="ExternalInput")
    wk = nc.dram_tensor("wk", [128, NDC, HL], BF16, kind="ExternalInput")
    wv = nc.dram_tensor("wv", [128, NDC, HL], BF16, kind="ExternalInput")
    wo = nc.dram_tensor("wo", [128, HL // 128, D], BF16, kind="ExternalInput")
    cosT = nc.dram_tensor("cosT", [HD, S], F32, kind="ExternalInput")
    sinT = nc.dram_tensor("sinT", [HD, S], F32, kind="ExternalInput")
    pmatT = nc.dram_tensor("pmatT", [HD, HD], BF16, kind="ExternalInput")
    maskT = nc.dram_tensor("maskT", [128, 512], BF16, kind="ExternalInput")
    onesd = nc.dram_tensor("onesd", [128, 128], BF16, kind="ExternalInput")
    out = nc.dram_tensor("out", [S, D], BF16, kind="ExternalOutput")

    with TileContext(nc) as tc:
        with (
            tc.tile_pool(name="consts", bufs=1) as consts,
            tc.tile_pool(name="resid", bufs=1) as resid,
            tc.tile_pool(name="psA", bufs=3, space="PSUM") as psA,
            tc.tile_pool(name="psB", bufs=3, space="PSUM") as psB,
            tc.tile_pool(name="psC", bufs=2, space="PSUM") as psC,
        ):
            # SBUF-resident q^T/k^T (per head) and v (per k-chunk), all bf16
            q_sb = [resid.tile([HD, S], BF16, name=f"qT{h}") for h in range(HPC)]
            k_sb = [resid.tile([HD, S], BF16, name=f"kT{h}") for h in range(HPC)]
            v_sb = resid.tile([128, NKC, HL], BF16, name="v_sb")
            ctxs = [resid.tile([128, S], BF16, name=f"ctxT{h}") for h in range(HPC)]

            pmat_sb = consts.tile([HD, HD], BF16, name="pmat_sb")
            mask_sb = consts.tile([128, 512], BF16, name="mask_sb")
            ones_sb = consts.tile([128, 128], BF16, name="ones_sb")
            ones_col = ones_sb[:, 0:1]
            gpwarm = consts.tile([128, 128], F32, name="gpwarm")

            # ---------------- phase 1: projections + RoPE ----------------
            with (
                tc.tile_pool(name="wpool", bufs=1) as wpool,
                tc.tile_pool(name="xtp", bufs=2) as xtp,
                tc.tile_pool(name="st1", bufs=2) as st1,
            ):
                cos_sb = wpool.tile([HD, S], F32, name="cos_sb")
                sin_sb = wpool.tile([HD, S], F32, name="sin_sb")

                # critical-path loads stay off the gpsimd queue (its library
                # load stalls it ~11us at boot). scalar: small consts + RoPE
                # tables; sync: wq (quartered so the first Q group can start
                # on quarter 0) then wk; vector: x blocks (x0 quartered);
                # gpsimd: warmup + wv (not needed until ~55us in).
                nc.scalar.dma_start(out=pmat_sb[:], in_=pmatT[:])
                nc.scalar.dma_start(out=cos_sb[:], in_=cosT[:])
                nc.scalar.dma_start(out=sin_sb[:], in_=sinT[:])
                nc.scalar.dma_start(out=mask_sb[:], in_=maskT[:])
                nc.scalar.dma_start(out=ones_sb[:], in_=onesd[:])

                w_sb = {}
                for nm, w_d in (("wq", wq), ("wk", wk), ("wv", wv)):
                    w_sb[nm] = wpool.tile([128, NDC, HL], BF16, name=f"{nm}_sb")

                def load_x_block(sb, quartered=False):
                    xt = xtp.tile([128, NDC, 512], BF16, tag="xt", name="xt")
                    if quartered:
                        for q in range(4):
                            nc.scalar.dma_start(
                                out=xt[:, q * 4:(q + 1) * 4, :],
                                in_=xT[:, sb, q * 4:(q + 1) * 4, :])
                    else:
                        nc.scalar.dma_start(out=xt[:], in_=xT[:, sb])
                    return xt

                for q in range(4):
                    nc.sync.dma_start(
                        out=w_sb["wq"][:, q * 4:(q + 1) * 4, :],
                        in_=wq[:, q * 4:(q + 1) * 4, :])
                x_cur = load_x_block(0, quartered=True)
                nc.sync.dma_start(out=w_sb["wk"][:], in_=wk[:])
                x_next = load_x_block(1)
                # warm up the GpSimd library load off the critical path
                nc.gpsimd.partition_broadcast(gpwarm[:], gpwarm[0:1, :])
                nc.gpsimd.dma_start(out=w_sb["wv"][:], in_=wv[:])

                finishers = deque()

                def emit_finisher():
                    kind, args = finishers.popleft()
                    if kind == "qk":
                        ps, qraw, dst, sl = args
                        rot = psB.tile([128, 512], F32, tag="b", name="rot")
                        _mm(nc, rot[:], pmat_sb[:], qraw[:], start=True, stop=True)
                        acos = st1.tile([128, 512], F32, tag="acos", name="acos")
                        nc.vector.tensor_mul(acos[:], ps[:], cos_sb[:, sl])
                        rsin = st1.tile([128, 512], F32, tag="rsin", name="rsin")
                        nc.vector.tensor_mul(rsin[:], rot[:], sin_sb[:, sl])
                        nc.vector.tensor_add(dst[:, sl], rsin[:], acos[:])
                    else:
                        ps, kc = args
                        nc.scalar.activation(v_sb[:, kc, :], ps[:], AF.Copy)

                for sb in range(NSB):
                    sl = slice(sb * 512, (sb + 1) * 512)
                    if sb > 0:
                        x_cur = x_next
                        if sb < NSB - 1:
                            x_next = load_x_block(sb + 1)

                    for wname, dst in (("wq", q_sb), ("wk", k_sb)):
                        w_t = w_sb[wname]
                        for h in range(HPC):
                            ps = psA.tile([128, 512], F32, tag="a", name="ps")
                            for dc in range(NDC):
                                _mm(nc, ps[:],
                                    w_t[:, dc, h * HD:(h + 1) * HD],
                                    x_cur[:, dc, :],
                                    start=(dc == 0), stop=(dc == NDC - 1))
                            qraw = st1.tile([128, 512], BF16, tag="qraw", name="qraw")
                            nc.scalar.activation(qraw[:], ps[:], AF.Copy)
                            finishers.append(("qk", (ps, qraw, dst[h], sl)))
                            if len(finishers) > 1:
                                emit_finisher()

                    for sc in range(4):
                        ps = psA.tile([128, 512], F32, tag="a", name="ps")
                        for dc in range(NDC):
                            _mm(nc, ps[:],
                                x_cur[:, dc, sc * 128:(sc + 1) * 128],
                                w_sb["wv"][:, dc, :],
                                start=(dc == 0), stop=(dc == NDC - 1))
                        finishers.append(("v", (ps, sb * 4 + sc)))
                        if len(finishers) > 1:
                            emit_finisher()
                while finishers:
                    emit_finisher()

            # ---------- phase 2+3: attention + output projection ----------
            with (
                tc.tile_pool(name="pp", bufs=8) as pp,
                tc.tile_pool(name="accp", bufs=3) as accp,
                tc.tile_pool(name="sm", bufs=3) as sm,
                tc.tile_pool(name="wop", bufs=1) as wop,
                tc.tile_pool(name="outp", bufs=4) as outp,
                tc.tile_pool(name="pvp", bufs=3) as pvp,
            ):
                wo_sb = wop.tile([128, HPC * D], BF16, name="wo_sb")
                nc.sync.dma_start(out=wo_sb[:], in_=wo[:])

                lagq = deque()   # (lps|None, pv, pt, vt, ncols, first, last)
                fin = deque()    # (h, qb, lps, pvs)
                pv_done = set()  # ids of pv tiles whose accumulation is emitted
                osb_flip = [0]

                def emit_lpv(job):
                    lps, pv, pt, vtc, ncols, first, last = job
                    if lps is not None:
                        # l accumulated on the PE, one ones-matmul per chunk
                        _mm(nc, lps[:, 512 - ncols:], ones_col, pt[:, :ncols],
                            start=first, stop=last)
                    _mm(nc, pv[:, 512 - ncols:], vtc, pt[:, :ncols],
                        start=first, stop=last)
                    if last:
                        # evacuate PSUM right away (ACT has slack) so the pv
                        # bank recycles without waiting on the softmax-
                        # normalization chain
                        pvs = pvp.tile([128, 512], BF16, tag="pvs", name="pvs")
                        nc.scalar.activation(pvs[:], pv[:], AF.Copy)
                        pv_done.add(id(pvs))
                        pv_sbuf[id(pv)] = pvs

                def emit_finalize(job):
                    h, qb, lps, pvs = job
                    lsb = sm.tile([1, 512], F32, tag="lsb", name="lsb")
                    nc.vector.tensor_copy(lsb[:], lps[:])
                    repsb = sm.tile([128, 512], F32, tag="repsb", name="repsb")
                    nc.gpsimd.partition_broadcast(repsb[:], lsb[:])
                    rcps = sm.tile([128, 512], F32, tag="rcps", name="rcps")
                    rcp = sm.tile([128, 512], F32, tag="rcp", name="rcp")
                    nc.vector.reciprocal_approx_accurate(rcp[:], repsb[:], rcps[:])
                    nc.vector.tensor_mul(ctxs[h][:, qb * 512:(qb + 1) * 512],
                                         pvs[:], rcp[:])

                def emit_outproj(job):
                    qb, half = job
                    for qc in range(4 * qb + 2 * half, 4 * qb + 2 * half + 2):
                        for db in range(D // 512):
                            ops = psA.tile([128, 512], F32, tag="a", name="ops")
                            for h in range(HPC):
                                _mm(nc, ops[:],
                                    ctxs[h][:, qc * 128:(qc + 1) * 128],
                                    wo_sb[:, h * D + db * 512: h * D + (db + 1) * 512],
                                    start=(h == 0), stop=(h == HPC - 1))
                            osb = outp.tile([128, 512], BF16, tag="osb", name="osb")
                            # alternate the PSUM evacuation between ACT and DVE
                            if osb_flip[0] % 2 == 0:
                                nc.scalar.activation(osb[:], ops[:], AF.Copy)
                            else:
                                nc.vector.tensor_copy(osb[:], ops[:])
                            osb_flip[0] += 1
                            nc.sync.dma_start(
                                out=out[qc * 128:(qc + 1) * 128,
                                        db * 512:(db + 1) * 512],
                                in_=osb[:])

                pv_sbuf = {}
                pending_l = deque()
                outproj_ready = [0] * NSB  # finalizes emitted per q-block
                outproj_q = deque()        # q-blocks whose ctx is complete

                def pop_finalize():
                    if fin and id(fin[0][3]) in pv_sbuf:
                        h0, qb0, lps0, pv0 = fin.popleft()
                        emit_finalize((h0, qb0, lps0, pv_sbuf.pop(id(pv0))))
                        qb = qb0
                        outproj_ready[qb] += 1
                        if outproj_ready[qb] == HPC:
                            outproj_q.append((qb, 0))
                            outproj_q.append((qb, 1))

                for h in range(HPC):
                    for qb in range(NSB):
                        nk = 4 * qb + 4
                        # alternate the l row-sum between PE (ones-matmul per
                        # chunk) and DVE (chunk accumulation) to balance load
                        l_on_pe = (h + qb) % 2 == 0
                        pv = psB.tile([128, 512], F32, tag="b", name="pv")
                        lps = psC.tile([1, 512], F32, tag="c", name="lps")
                        acc = None
                        if not l_on_pe:
                            acc = accp.tile([128, 512], BF16, tag="acc", name="acc")
                        for kc in range(nk):
                            j = kc - 4 * qb
                            ncols = 512 if j < 0 else 512 - 128 * j
                            sps = psA.tile([128, 512], F32, tag="a", name="sps")
                            _mm(nc, sps[:, :ncols],
                                k_sb[h][:, kc * 128:(kc + 1) * 128],
                                q_sb[h][:, qb * 512 + 512 - ncols:(qb + 1) * 512],
                                start=True, stop=True)
                            pt = pp.tile([128, 512], BF16, tag="pt", name="pt")
                            nc.scalar.activation(pt[:, :ncols], sps[:, :ncols], AF.Exp)
                            if j >= 0:
                                nc.vector.tensor_mul(pt[:, :ncols], pt[:, :ncols],
                                                     mask_sb[:, :ncols])
                            if not l_on_pe:
                                if kc == 0:
                                    nc.vector.tensor_copy(acc[:], pt[:])
                                else:
                                    nc.vector.tensor_add(acc[:, 512 - ncols:],
                                                         acc[:, 512 - ncols:],
                                                         pt[:, :ncols])
                            lagq.append((lps if l_on_pe else None, pv, pt,
                                         v_sb[:, kc, h * HD:(h + 1) * HD],
                                         ncols, kc == 0, kc == nk - 1))
                            while len(lagq) > 2:
                                emit_lpv(lagq.popleft())
                            if kc % 2 == 1:
                                if pending_l:
                                    plps, pacc = pending_l.popleft()
                                    _mm(nc, plps[:], ones_col, pacc[:],
                                        start=True, stop=True)
                                elif outproj_q:
                                    emit_outproj(outproj_q.popleft())
                                else:
                                    pop_finalize()
                        if not l_on_pe:
                            # defer the row-sum matmul into the next window's
                            # stream so the PE never waits on DVE's last add
                            pending_l.append((lps, acc))
                        fin.append((h, qb, lps, pv))
                while lagq:
                    emit_lpv(lagq.popleft())
                while pending_l:
                    plps, pacc = pending_l.popleft()
                    _mm(nc, plps[:], ones_col, pacc[:], start=True, stop=True)
                while fin:
                    pop_finalize()
                    while outproj_q:
                        emit_outproj(outproj_q.popleft())

    nc.compile()
    return nc


_NC_CACHE = None


def _get_nc():
    global _NC_CACHE
    if _NC_CACHE is None:
        _NC_CACHE = _build()
    return _NC_CACHE


def _host_tables():
    # Replicate reference RoPE tables in float32 arithmetic, transposed.
    inv_freq = np.float32(1.0) / np.power(
        np.float32(ROPE_THETA), np.arange(0, HD, 2).astype(np.float32) / np.float32(HD)
    )
    pos = np.arange(S, dtype=np.float32)
    freqs = pos[:, None] * inv_freq[None, :]
    angles = np.concatenate([freqs, freqs], axis=1)  # [S, HD]
    cos_t = np.ascontiguousarray(np.cos(angles).astype(np.float32).T)  # [HD, S]
    sin_t = np.ascontiguousarray(np.sin(angles).astype(np.float32).T)
    # rotate_half as a left-multiply matrix P: (P q)[2i] = -q[2i+1], [2i+1] = q[2i].
    # matmul computes lhsT.T @ rhs, so feed P.T.
    pmat = np.zeros((HD, HD), dtype=np.float32)
    for i in range(HD // 2):
        pmat[2 * i, 2 * i + 1] = -1.0
        pmat[2 * i + 1, 2 * i] = 1.0
    pmat_t = np.ascontiguousarray(pmat.T).astype(BNP)
    mask = (np.arange(128)[:, None] <= np.arange(512)[None, :]).astype(BNP)
    return cos_t, sin_t, pmat_t, mask


_ONES = np.ones((128, 128), dtype=BNP)


def kernel(x, Wq, Wk, Wv, Wo):
    x = np.asarray(x, dtype=np.float32)
    Wq = np.asarray(Wq, dtype=np.float32)
    Wk = np.asarray(Wk, dtype=np.float32)
    Wv = np.asarray(Wv, dtype=np.float32)
    Wo = np.asarray(Wo, dtype=np.float32)

    results = _run_device(x, Wq, Wk, Wv, Wo)

    out = np.empty((B, S, D), dtype=np.float32)
    for b in range(B):
        acc = results[b * CORES_PER_BATCH]["out"].astype(np.float32)
        for i in range(1, CORES_PER_BATCH):
            acc = acc + results[b * CORES_PER_BATCH + i]["out"].astype(np.float32)
        out[b] = acc
    return out


def _make_in_maps(x, Wq, Wk, Wv, Wo):
    cos_t, sin_t, pmat_t, mask = _host_tables()
    scale = np.float32(1.0 / math.sqrt(HD))
    def dev_w(w):  # [D, HL_slice] -> [128, NDC, hl]
        return np.ascontiguousarray(
            w.reshape(NDC, 128, -1).transpose(1, 0, 2)).astype(BNP)

    wq_scaled = (Wq * scale).astype(np.float32)
    xTb = [
        np.ascontiguousarray(
            x[b].T.reshape(NDC, 128, NSB, 512).transpose(1, 2, 0, 3)).astype(BNP)
        for b in range(B)
    ]
    in_maps = []
    for c in range(N_CORES):
        b = c // CORES_PER_BATCH
        g = c % CORES_PER_BATCH
        hs = slice(g * HL, (g + 1) * HL)
        in_maps.append({
            "xT": xTb[b],
            "wq": dev_w(wq_scaled[:, hs]),
            "wk": dev_w(Wk[:, hs]),
            "wv": dev_w(Wv[:, hs]),
            "wo": np.ascontiguousarray(
                Wo[hs, :].reshape(HL // 128, 128, D).transpose(1, 0, 2)).astype(BNP),
            "cosT": cos_t,
            "sinT": sin_t,
            "pmatT": pmat_t,
            "maskT": mask,
            "onesd": _ONES,
        })
    return in_maps


def _run_device(x, Wq, Wk, Wv, Wo, trace=False):
    nc = _get_nc()
    in_maps = _make_in_maps(x, Wq, Wk, Wv, Wo)
    res = run_bass_kernel_spmd(nc, in_maps, core_ids=list(range(N_CORES)), trace=trace)
    if trace:
        return res
    return res.results


def run_traced(x, Wq, Wk, Wv, Wo):
    """Run with NTFF tracing; returns (full_output, BassKernelResults)."""
    res = _run_device(np.asarray(x, np.float32), np.asarray(Wq, np.float32),
                      np.asarray(Wk, np.float32), np.asarray(Wv, np.float32),
                      np.asarray(Wo, np.float32), trace=True)
    out = np.empty((B, S, D), dtype=np.float32)
    for b in range(B):
        acc = res.results[b * CORES_PER_BATCH]["out"].astype(np.float32)
        for i in range(1, CORES_PER_BATCH):
            acc = acc + res.results[b * CORES_PER_BATCH + i]["out"].astype(np.float32)
        out[b] = acc
    return out, res

